# revision 9
# baseline (speedup 1.0000x reference)
"""Distributed multi-head attention for TRN2 (8 NeuronCores).

Reference computation (per problem spec):
    q = (query @ Wq.T + bq)  -> [B,T,H,Hd] -> heads
    k = (key_  @ Wk.T + bk)
    v = (value @ Wv.T + bv)
    out = softmax(q k^T * Hd^-0.5) v   (full T x S scores)
    out = out @ Wo.T + bo

Sharding: 8 cores = B(2) x T-quarters(4).  Each core computes all 8 heads
for its 1024 query rows (k/v projections are recomputed per core; no
collectives needed).

Device algorithm (per core, transposed-flash layout):
  - inputs pre-transposed on host: qT [D, Tl], kT [D, S], vT_aug [D+1, S]
  - projections produce qT_p/kT_p [D, *] (d on partitions) and
    v_aug [S, 8*65] where each head has 64 v-columns plus a ones column
    (ones + biases come from an augmented contraction row, host-built)
  - scores transposed: sT[s_tile 128, t 512] = kT_p^T-slice @ qT_p-slice (K=64)
  - exp on ScalarE with scale folded in; groups of 3 PSUM banks per
    activation op to amortize ACT fixed overhead
  - PV matmul: lhsT = v_aug slice [s 128, 65], rhs = expT; PSUM accumulates
    [65, 512]; row 64 is the softmax denominator (from the ones column)
  - normalize: reciprocal of row 64, partition_broadcast, one multiply
  - out-proj: lhsT = normalized attT [64, t-tile], rhs = Wo.T rows; 8 heads
    accumulate into one PSUM tile; bias bo is added on host.
"""

import sys

sys.path.insert(0, "/opt/trn_rl_repo")

import numpy as np

N_CORES = 8
B, T, D, H, HD = 2, 4096, 512, 8, 64
SCALE = HD ** -0.5
TQ = 4                # T-quarters per batch
T_LOC = T // TQ       # 1024 query rows per core
S = T                 # kv sequence length
KC = D // 128         # 4 contraction chunks of 128
NS = S // 128         # 32 s-tiles
VW = H * (HD + 1)     # 520: v_aug width (per head: 64 v cols + ones col)
EXPG = 3              # s-tiles per exp group (3 PSUM banks per ACT op)

_cache = {}


def _build():
    import concourse.bacc as bacc
    import concourse.mybir as mybir
    import concourse.tile as tile

    dt = mybir.dt
    f32, bf16 = dt.float32, dt.bfloat16
    AF = mybir.ActivationFunctionType

    nc = bacc.Bacc("TRN2", target_bir_lowering=False, debug=False,
                   num_devices=N_CORES)

    qT_d = nc.dram_tensor("qT", [D, T_LOC], bf16, kind="ExternalInput").ap()
    kT_d = nc.dram_tensor("kT", [D, S], bf16, kind="ExternalInput").ap()
    vTa_d = nc.dram_tensor("vTa", [D + 1, S], bf16, kind="ExternalInput").ap()
    wqT_d = nc.dram_tensor("wqT", [D, D], bf16, kind="ExternalInput").ap()
    wkT_d = nc.dram_tensor("wkT", [D, D], bf16, kind="ExternalInput").ap()
    wvA_d = nc.dram_tensor("wvA", [D + 1, VW], bf16, kind="ExternalInput").ap()
    woT_d = nc.dram_tensor("woT", [64, H * 512], bf16, kind="ExternalInput").ap()
    bq_d = nc.dram_tensor("bq2", [128, KC], f32, kind="ExternalInput").ap()
    bk_d = nc.dram_tensor("bk2", [128, KC], f32, kind="ExternalInput").ap()
    out_d = nc.dram_tensor("out", [T_LOC, D], f32, kind="ExternalOutput").ap()

    with tile.TileContext(nc) as tc:
        with tc.tile_pool(name="persist", bufs=1) as pp:
            # persistent SBUF tensors
            wq_sb = pp.tile([128, KC * 512], bf16, tag="wq")
            wk_sb = pp.tile([128, KC * 512], bf16, tag="wk")
            wv_sb = pp.tile([128, KC * VW], bf16, tag="wv")
            wvL_sb = pp.tile([1, VW], bf16, tag="wvL")
            wo_sb = pp.tile([64, H * 512], bf16, tag="wo")
            bq_sb = pp.tile([128, KC], f32, tag="bq")
            bk_sb = pp.tile([128, KC], f32, tag="bk")
            qTp = pp.tile([128, KC * T_LOC], bf16, tag="qTp")
            kTp = pp.tile([128, KC * S], bf16, tag="kTp")
            vA = pp.tile([128, NS * VW], bf16, tag="vA")

            # ---------------- phase A: projections ----------------
            with tc.tile_pool(name="inp", bufs=1) as ip, \
                 tc.tile_pool(name="papsum", bufs=3, space="PSUM") as pap:
                qin = ip.tile([128, KC * T_LOC], bf16, tag="qin")
                kin = ip.tile([128, KC * S], bf16, tag="kin")
                vin = ip.tile([128, KC * S], bf16, tag="vin")
                vinL = ip.tile([1, S], bf16, tag="vinL")

                nc.sync.dma_start(bq_sb[:, :], bq_d[:, :])
                nc.sync.dma_start(bk_sb[:, :], bk_d[:, :])
                for ki in range(KC):
                    r = slice(ki * 128, (ki + 1) * 128)
                    nc.sync.dma_start(wq_sb[:, ki * 512:(ki + 1) * 512], wqT_d[r, :])
                    nc.sync.dma_start(qin[:, ki * T_LOC:(ki + 1) * T_LOC], qT_d[r, :])
                for ki in range(KC):
                    r = slice(ki * 128, (ki + 1) * 128)
                    nc.sync.dma_start(wk_sb[:, ki * 512:(ki + 1) * 512], wkT_d[r, :])
                    nc.sync.dma_start(kin[:, ki * S:(ki + 1) * S], kT_d[r, :])
                for ki in range(KC):
                    r = slice(ki * 128, (ki + 1) * 128)
                    nc.sync.dma_start(wv_sb[:, ki * VW:(ki + 1) * VW], wvA_d[r, :])
                    nc.sync.dma_start(vin[:, ki * S:(ki + 1) * S], vTa_d[r, :])
                nc.sync.dma_start(vinL[:, :], vTa_d[D:D + 1, :])
                nc.sync.dma_start(wvL_sb[:, :], wvA_d[D:D + 1, :])
                nc.sync.dma_start(wo_sb[:, :], woT_d[:, :])

                # qT_p[d, t] = sum_i WqT[i, d] * queryT[i, t]  (+bq per-partition)
                for mi in range(KC):
                    for tn in range(T_LOC // 512):
                        psq = pap.tile([128, 512], f32, tag="qk", bufs=4)
                        for ki in range(KC):
                            nc.tensor.matmul(
                                psq[:, :],
                                lhsT=wq_sb[:, ki * 512 + mi * 128: ki * 512 + (mi + 1) * 128],
                                rhs=qin[:, ki * T_LOC + tn * 512: ki * T_LOC + (tn + 1) * 512],
                                start=(ki == 0), stop=(ki == KC - 1))
                        nc.vector.tensor_scalar_add(
                            qTp[:, mi * T_LOC + tn * 512: mi * T_LOC + (tn + 1) * 512],
                            psq[:, :], bq_sb[:, mi:mi + 1])

                # kT_p[d, s]
                for mi in range(KC):
                    for sn in range(S // 512):
                        psk = pap.tile([128, 512], f32, tag="qk", bufs=4)
                        for ki in range(KC):
                            nc.tensor.matmul(
                                psk[:, :],
                                lhsT=wk_sb[:, ki * 512 + mi * 128: ki * 512 + (mi + 1) * 128],
                                rhs=kin[:, ki * S + sn * 512: ki * S + (sn + 1) * 512],
                                start=(ki == 0), stop=(ki == KC - 1))
                        nc.vector.tensor_scalar_add(
                            kTp[:, mi * S + sn * 512: mi * S + (sn + 1) * 512],
                            psk[:, :], bk_sb[:, mi:mi + 1])

                # v_aug[s, c] = sum_i valueT_aug[i, s] * WvA[i, c]
                # (c = h*65+j: j<64 v-dims with +bv folded; j=64 ones col)
                for si in range(NS):
                    psv = pap.tile([128, VW], f32, tag="v", bufs=2)
                    for lo, hi in ((0, 512), (512, VW)):
                        for ki in range(KC):
                            nc.tensor.matmul(
                                psv[:, lo:hi],
                                lhsT=vin[:, ki * S + si * 128: ki * S + (si + 1) * 128],
                                rhs=wv_sb[:, ki * VW + lo: ki * VW + hi],
                                start=(ki == 0), stop=False)
                        nc.tensor.matmul(
                            psv[:, lo:hi],
                            lhsT=vinL[:, si * 128:(si + 1) * 128],
                            rhs=wvL_sb[:, lo:hi],
                            start=False, stop=True)
                    nc.vector.tensor_copy(vA[:, si * VW:(si + 1) * VW], psv[:, :])

            # ---------------- phase B: attention ----------------
            with tc.tile_pool(name="pvpsum", bufs=2, space="PSUM") as pvp, \
                 tc.tile_pool(name="spsum", bufs=2, space="PSUM") as sp, \
                 tc.tile_pool(name="work", bufs=2) as wp:
                for tn in range(T_LOC // 512):
                    raw_sb = wp.tile([64, H * 512], bf16, tag="raw")
                    for h in range(H):
                        mi, po = h // 2, (h % 2) * 64
                        pv = pvp.tile([65, 512], f32, tag="pv")
                        ngroups = (NS + EXPG - 1) // EXPG
                        for g in range(ngroups):
                            g0 = g * EXPG
                            gsz = min(EXPG, NS - g0)
                            sc = sp.tile([128, EXPG * 512], f32, tag="sc")
                            for j in range(gsz):
                                si = g0 + j
                                nc.tensor.matmul(
                                    sc[:, j * 512:(j + 1) * 512],
                                    lhsT=kTp[po:po + 64, mi * S + si * 128: mi * S + (si + 1) * 128],
                                    rhs=qTp[po:po + 64, mi * T_LOC + tn * 512: mi * T_LOC + (tn + 1) * 512],
                                    start=True, stop=True)
                            exp_t = wp.tile([128, EXPG * 512], bf16, tag="exp", bufs=3)
                            nc.scalar.activation(
                                exp_t[:, 0:gsz * 512], sc[:, 0:gsz * 512],
                                AF.Exp, scale=float(SCALE))
                            for j in range(gsz):
                                si = g0 + j
                                nc.tensor.matmul(
                                    pv[:, :],
                                    lhsT=vA[:, si * VW + h * 65: si * VW + (h + 1) * 65],
                                    rhs=exp_t[:, j * 512:(j + 1) * 512],
                                    start=(si == 0), stop=(si == NS - 1))
                        # normalize: raw[0:64] / denom(row 64)
                        recip_t = wp.tile([1, 512], f32, tag="recip")
                        nc.vector.reciprocal(recip_t[:, :], pv[64:65, :])
                        bc_t = wp.tile([64, 512], f32, tag="bc")
                        nc.gpsimd.partition_broadcast(bc_t[:, :], recip_t[:, :])
                        nc.vector.tensor_mul(
                            raw_sb[:, h * 512:(h + 1) * 512], pv[0:64, :], bc_t[:, :])

                    # out projection for this t-chunk (4 t-tiles of 128)
                    for tt in range(4):
                        pso = sp.tile([128, EXPG * 512], f32, tag="sc")
                        for h in range(H):
                            nc.tensor.matmul(
                                pso[:, 0:512],
                                lhsT=raw_sb[:, h * 512 + tt * 128: h * 512 + (tt + 1) * 128],
                                rhs=wo_sb[:, h * 512:(h + 1) * 512],
                                start=(h == 0), stop=(h == H - 1))
                        out_t = wp.tile([128, 512], f32, tag="out")
                        nc.vector.tensor_copy(out_t[:, :], pso[:, 0:512])
                        nc.sync.dma_start(
                            out_d[tn * 512 + tt * 128: tn * 512 + (tt + 1) * 128, :],
                            out_t[:, :])

    nc.compile()
    return nc


def get_nc():
    if "nc" not in _cache:
        _cache["nc"] = _build()
    return _cache["nc"]


def host_prep(query, key_, value, Wq, bq, Wk, bk, Wv, bv, Wo, bo):
    """Build the 8 per-core input maps (all numpy, bf16 except biases)."""
    import ml_dtypes
    bf16 = ml_dtypes.bfloat16

    def f(x):
        return np.ascontiguousarray(np.asarray(x, dtype=np.float32))

    query, key_, value = f(query), f(key_), f(value)
    Wq, Wk, Wv, Wo = f(Wq), f(Wk), f(Wv), f(Wo)
    bq, bk, bv, bo = f(bq), f(bk), f(bv), f(bo)

    wqT = np.ascontiguousarray(Wq.T).astype(bf16)
    wkT = np.ascontiguousarray(Wk.T).astype(bf16)
    woT = np.concatenate(
        [Wo.T[h * 64:(h + 1) * 64, :] for h in range(H)], axis=1).astype(bf16)
    # v-projection augmented weights: [D+1, H*(HD+1)]
    wvA = np.zeros((D + 1, VW), dtype=np.float32)
    for h in range(H):
        wvA[:D, h * 65: h * 65 + 64] = Wv[h * 64:(h + 1) * 64, :].T
        wvA[D, h * 65: h * 65 + 64] = bv[h * 64:(h + 1) * 64]
        wvA[D, h * 65 + 64] = 1.0
    wvA = wvA.astype(bf16)
    # per-partition bias layout [128, KC]: col mi = bias[mi*128 : (mi+1)*128]
    bq2 = np.ascontiguousarray(bq.reshape(KC, 128).T)
    bk2 = np.ascontiguousarray(bk.reshape(KC, 128).T)

    in_maps = []
    for c in range(N_CORES):
        b, tq = c // TQ, c % TQ
        qT = np.ascontiguousarray(
            query[b, tq * T_LOC:(tq + 1) * T_LOC, :].T).astype(bf16)
        kT = np.ascontiguousarray(key_[b].T).astype(bf16)
        vTa = np.concatenate(
            [value[b].T, np.ones((1, S), np.float32)], axis=0).astype(bf16)
        in_maps.append({
            "qT": qT, "kT": kT, "vTa": vTa,
            "wqT": wqT, "wkT": wkT, "wvA": wvA, "woT": woT,
            "bq2": bq2, "bk2": bk2,
        })
    return in_maps


def gather(results, bo):
    """Assemble full [B, T, D] output from per-core results."""
    out = np.empty((B, T, D), dtype=np.float32)
    for c in range(N_CORES):
        b, tq = c // TQ, c % TQ
        out[b, tq * T_LOC:(tq + 1) * T_LOC, :] = results[c]["out"]
    out += np.asarray(bo, dtype=np.float32)
    return out


def kernel(query, key_, value, Wq, bq, Wk, bk, Wv, bv, Wo, bo):
    from concourse.bass_utils import run_bass_kernel_spmd

    nc = get_nc()
    in_maps = host_prep(query, key_, value, Wq, bq, Wk, bk, Wv, bv, Wo, bo)
    res = run_bass_kernel_spmd(nc, in_maps, core_ids=list(range(N_CORES)))
    _cache["last_result"] = res
    return gather(res.results, bo)
